# revision 11
# baseline (speedup 1.0000x reference)
"""MoE layer (top-2 of 8 experts) on 8 TRN2 NeuronCores, expert-parallel.

Sharding: expert e lives on core e. The gate (x @ Wg + bg, top-2, softmax)
is computed on the host as part of routing/sharding: each core receives only
the tokens routed to its expert (gathered + transposed + bf16) and its
expert's weights. The device runs the 3-layer expert MLP (bf16 matmuls,
fp32 accumulate, fused relu+bias on ScalarE, b3 via a rank-1 matmul into
PSUM) producing Y^T = [O, C] per core. The host applies the per-token gate
weight and scatter-adds the two expert outputs per token back to [B, L, O].

Layout notes (all activations stay feature-major so weight matrices are the
stationary matmul operand with no transposes anywhere):
  stage1: H1^T[h, c] = W1^T @ X^T    (contract D,  8 k-tiles)
  stage2: H2^T[g, c] = W2^T @ H1^T   (contract H, 16 k-tiles)
  stage3: Y^T[o, c]  = W3^T @ H2^T   (contract H, 16 k-tiles) + b3 x 1^T

Each instruction carries at most ONE hardware sync-wait on this toolchain:
biases are pre-copied by ScalarE so relu ACTs only wait on PE; stage-3
results are DMA'd straight from PSUM; the rank-1 b3 matmul (start=True)
absorbs the PSUM-recycle wait so the k-loop matmuls only wait on ACT.
"""

import os

import numpy as np
import ml_dtypes

import concourse.bass as bass
import concourse.bacc as bacc
import concourse.mybir as mybir
import concourse.tile as tile
from concourse.bass_utils import run_bass_kernel_spmd

B, L, D, H, O, E, K = 4, 2048, 1024, 2048, 1024, 8, 2
P = 128
KD, KH, KO = D // P, H // P, O // P
CHUNK = 512
N_CORES = 8

BF16 = mybir.dt.bfloat16
F32 = mybir.dt.float32

TRACE = os.environ.get("MOE_BASS_TRACE") == "1"
LAST_RESULT = None

_nc_cache = {}


def _build(C):
    nc = bacc.Bacc()
    xT = nc.dram_tensor("xT", [D, C], BF16, kind="ExternalInput")
    # W1/W2 arrive host-prepacked as [col_pair, k, P, 2P] so the kernel can
    # stream them column-pair-major (consumption order) with fully
    # contiguous 64 KB DMA reads.
    w1 = nc.dram_tensor("w1", [KH // 2, KD, P, 2 * P], BF16, kind="ExternalInput")
    w2 = nc.dram_tensor("w2", [KH // 2, KH, P, 2 * P], BF16, kind="ExternalInput")
    w3 = nc.dram_tensor("w3", [H, O], BF16, kind="ExternalInput")
    b1 = nc.dram_tensor("b1", [KH, P, 1], F32, kind="ExternalInput")
    b2 = nc.dram_tensor("b2", [KH, P, 1], F32, kind="ExternalInput")
    out = nc.dram_tensor("out", [O, C], F32, kind="ExternalOutput")

    # Balanced chunk widths (<= CHUNK each) so no chunk pays the small-N
    # per-matmul overhead premium.
    n_chunks = (C + CHUNK - 1) // CHUNK
    base, rem = divmod(C, n_chunks)
    widths = [base + (1 if i < rem else 0) for i in range(n_chunks)]
    starts = [sum(widths[:i]) for i in range(n_chunks)]
    relu = mybir.ActivationFunctionType.Relu

    with tile.TileContext(nc) as tc:
        with (
            tc.tile_pool(name="weights", bufs=1) as wp,
            tc.tile_pool(name="acts", bufs=1) as hp,
            tc.tile_pool(name="xin", bufs=2) as xp,
            tc.tile_pool(name="yout", bufs=3) as yp,
            tc.tile_pool(name="psum", bufs=2, space="PSUM") as pp,
        ):
            def dma_x(ci):
                t = xp.tile([P, KD, widths[ci]], BF16, tag="x")
                c0 = starts[ci]
                xr = xT[:, c0:c0 + widths[ci]].rearrange(
                    "(k p) c -> p k c", p=P
                )
                for k in range(KD):
                    nc.sync.dma_start(t[:, k, :], xr[:, k, :])
                return t

            # Short PE warmup on memset tiles: burns the cold HAM window
            # (half-rate PE clock) on dummy work before real data lands.
            wu_w = wp.tile([P, P], BF16, tag="wuw")
            wu_x = wp.tile([P, 256], BF16, tag="wux")
            nc.vector.memset(wu_w, 1.0)
            nc.vector.memset(wu_x, 1.0)
            wu_ps = pp.tile([P, 256], F32, tag="wups", bufs=1)
            for _ in range(10):
                nc.tensor.matmul(wu_ps, wu_w, wu_x, start=True, stop=True)

            # DMA issue order = consumption order: small biases and the
            # first chunks of x go first, then W1/W2/W3 k-tiles, so the
            # first matmuls start as soon as W1[k0] + x[0] land instead of
            # after the full 17 MB weight prologue.
            b1_dma = wp.tile([P, KH], F32, tag="b1d")
            nc.sync.dma_start(b1_dma, b1[:].rearrange("n p one -> p (n one)"))
            b2_dma = wp.tile([P, KH], F32, tag="b2d")
            nc.sync.dma_start(b2_dma, b2[:].rearrange("n p one -> p (n one)"))
            x_tiles = {0: dma_x(0)}
            # W1/W2 stream column-major (the order the ho/go loops consume
            # them) so chunk 0's compute never waits on a late k-tile.
            w1_k = [wp.tile([P, H], BF16, name=f"w1_{k}", tag=f"w1_{k}") for k in range(KD)]
            for hp2 in range(KH // 2):
                cs = slice(hp2 * 2 * P, (hp2 + 1) * 2 * P)
                for k in range(KD):
                    nc.sync.dma_start(w1_k[k][:, cs], w1[hp2, k])
            if n_chunks > 1:
                x_tiles[1] = dma_x(1)
            w2_k = [wp.tile([P, H], BF16, name=f"w2_{k}", tag=f"w2_{k}") for k in range(KH)]
            for gp in range(KH // 2):
                cs = slice(gp * 2 * P, (gp + 1) * 2 * P)
                for k in range(KH):
                    nc.sync.dma_start(w2_k[k][:, cs], w2[gp, k])
            w3_k = []
            for k in range(KH):
                t = wp.tile([P, O], BF16, tag=f"w3_{k}")
                nc.sync.dma_start(t, w3[k * P:(k + 1) * P, :])
                w3_k.append(t)

            for ci in range(n_chunks):
                c0 = starts[ci]
                cw = widths[ci]
                x_sb = x_tiles.pop(ci)
                if ci + 2 < n_chunks:
                    x_tiles[ci + 2] = dma_x(ci + 2)
                h1 = hp.tile([P, KH, cw], BF16, tag="h1")
                for ho in range(KH):
                    ps = pp.tile([P, cw], F32, tag="ps1")
                    for k in range(KD):
                        nc.tensor.matmul(
                            ps,
                            w1_k[k][:, ho * P:(ho + 1) * P],
                            x_sb[:, k, :],
                            start=(k == 0),
                            stop=(k == KD - 1),
                        )
                    nc.scalar.activation(
                        h1[:, ho, :], ps, relu, bias=b1_dma[:, ho:ho + 1]
                    )
                h2 = hp.tile([P, KH, cw], BF16, tag="h2")
                for go in range(KH):
                    ps = pp.tile([P, cw], F32, tag="ps2")
                    for k in range(KH):
                        nc.tensor.matmul(
                            ps,
                            w2_k[k][:, go * P:(go + 1) * P],
                            h1[:, k, :],
                            start=(k == 0),
                            stop=(k == KH - 1),
                        )
                    nc.scalar.activation(
                        h2[:, go, :], ps, relu, bias=b2_dma[:, go:go + 1]
                    )
                for oo in range(KO):
                    ps = pp.tile([P, cw], F32, tag="ps3")
                    for k in range(KH):
                        nc.tensor.matmul(
                            ps,
                            w3_k[k][:, oo * P:(oo + 1) * P],
                            h2[:, k, :],
                            start=(k == 0),
                            stop=(k == KH - 1),
                        )
                    y = yp.tile([P, cw], F32, tag="y")
                    nc.vector.tensor_copy(y, ps)
                    nc.sync.dma_start(
                        out[oo * P:(oo + 1) * P, c0:c0 + cw], y
                    )
    nc.compile()
    return nc


def kernel(x, W1, b1, W2, b2, W3, b3, Wg, bg):
    global LAST_RESULT
    x = np.asarray(x, dtype=np.float32)
    xf = x.reshape(B * L, D)

    # Gate on host (routing): logits in f64 for a faithful top-2 selection.
    logits = xf.astype(np.float64) @ np.asarray(Wg, np.float64) + np.asarray(
        bg, np.float64
    )
    order = np.argsort(-logits, axis=1, kind="stable")[:, :K]
    topv = np.take_along_axis(logits, order, 1).astype(np.float32)
    mx = topv.max(1, keepdims=True)
    ex = np.exp(topv - mx)
    w = (ex / ex.sum(1, keepdims=True)).astype(np.float32)

    toks, wgts, counts = [], [], []
    for e in range(E):
        s0 = np.nonzero(order[:, 0] == e)[0]
        s1 = np.nonzero(order[:, 1] == e)[0]
        toks.append(np.concatenate([s0, s1]))
        wgts.append(np.concatenate([w[s0, 0], w[s1, 1]]).astype(np.float32))
        counts.append(len(toks[-1]))

    # Exact capacity: every core's kernel runs C token slots; shorter
    # experts are padded with token 0 and their extra rows dropped on the
    # host. Pad to a multiple of 4 to keep DMA rows 8-byte aligned.
    C = max(256, -(-max(counts) // 4) * 4)
    nc = _nc_cache.get(C)
    if nc is None:
        nc = _build(C)
        _nc_cache[C] = nc

    bf = ml_dtypes.bfloat16
    in_maps = []
    for e in range(E):
        n = counts[e]
        pad_tok = np.zeros(C, np.int64)
        pad_tok[:n] = toks[e]
        in_maps.append({
            "xT": np.ascontiguousarray(xf[pad_tok].T.astype(bf)),
            "w1": np.ascontiguousarray(
                np.asarray(W1[e], np.float32).astype(bf)
                .reshape(KD, P, KH // 2, 2 * P).transpose(2, 0, 1, 3)
            ),
            "w2": np.ascontiguousarray(
                np.asarray(W2[e], np.float32).astype(bf)
                .reshape(KH, P, KH // 2, 2 * P).transpose(2, 0, 1, 3)
            ),
            "w3": np.asarray(W3[e], np.float32).astype(bf),
            "b1": np.ascontiguousarray(
                np.asarray(b1[e], np.float32).reshape(KH, P, 1)
            ),
            "b2": np.ascontiguousarray(
                np.asarray(b2[e], np.float32).reshape(KH, P, 1)
            ),
        })

    res = run_bass_kernel_spmd(
        nc, in_maps, core_ids=list(range(N_CORES)), trace=TRACE
    )
    LAST_RESULT = res

    # Combine: weight each expert's [O, C] output by the token's gate weight
    # and scatter-add; every token has exactly K=2 entries.
    y_all = np.concatenate(
        [
            np.asarray(res.results[e]["out"])[:, :counts[e]].T * wgts[e][:, None]
            for e in range(E)
        ],
        axis=0,
    )
    idx_all = np.concatenate(toks)
    srt = np.argsort(idx_all, kind="stable")
    ys = y_all[srt]
    out_flat = ys[0::2] + ys[1::2]
    # b3 is applied here rather than on-device: out += sum_k w_k * b3[e_k]
    b3f = np.asarray(b3, np.float32)
    out_flat += w[:, 0, None] * b3f[order[:, 0]] + w[:, 1, None] * b3f[order[:, 1]]
    out = out_flat.reshape(B, L, O).astype(np.float32)

    usage = w.reshape(B, L, K).mean(axis=1)
    lbl = np.float32(usage.var(axis=-1, ddof=1).mean())
    return out, lbl


# revision 12
# speedup vs baseline: 1.1087x; 1.1087x over previous
"""MoE layer (top-2 of 8 experts) on 8 TRN2 NeuronCores, expert-parallel.

Sharding: expert e lives on core e. The gate (x @ Wg + bg, top-2, softmax)
is computed on the host as part of routing/sharding: each core receives only
the tokens routed to its expert (gathered + transposed + bf16) and its
expert's weights. The device runs the 3-layer expert MLP (bf16 matmuls,
fp32 accumulate, fused relu+bias on ScalarE, b3 via a rank-1 matmul into
PSUM) producing Y^T = [O, C] per core. The host applies the per-token gate
weight and scatter-adds the two expert outputs per token back to [B, L, O].

Layout notes (all activations stay feature-major so weight matrices are the
stationary matmul operand with no transposes anywhere):
  stage1: H1^T[h, c] = W1^T @ X^T    (contract D,  8 k-tiles)
  stage2: H2^T[g, c] = W2^T @ H1^T   (contract H, 16 k-tiles)
  stage3: Y^T[o, c]  = W3^T @ H2^T   (contract H, 16 k-tiles) + b3 x 1^T

Each instruction carries at most ONE hardware sync-wait on this toolchain:
biases are pre-copied by ScalarE so relu ACTs only wait on PE; stage-3
results are DMA'd straight from PSUM; the rank-1 b3 matmul (start=True)
absorbs the PSUM-recycle wait so the k-loop matmuls only wait on ACT.
"""

import os

import numpy as np
import ml_dtypes

import concourse.bass as bass
import concourse.bacc as bacc
import concourse.mybir as mybir
import concourse.tile as tile
from concourse.bass_utils import run_bass_kernel_spmd

B, L, D, H, O, E, K = 4, 2048, 1024, 2048, 1024, 8, 2
P = 128
KD, KH, KO = D // P, H // P, O // P
CHUNK = 512
N_CORES = 8

BF16 = mybir.dt.bfloat16
F32 = mybir.dt.float32

TRACE = os.environ.get("MOE_BASS_TRACE") == "1"
LAST_RESULT = None

_nc_cache = {}


def _build(C):
    nc = bacc.Bacc()
    xF = nc.dram_tensor("xF", [P, KD * C], BF16, kind="ExternalInput")
    w1 = nc.dram_tensor("w1", [D, H], BF16, kind="ExternalInput")
    w2 = nc.dram_tensor("w2", [H, H], BF16, kind="ExternalInput")
    w3 = nc.dram_tensor("w3", [H, O], BF16, kind="ExternalInput")
    b1 = nc.dram_tensor("b1", [KH, P, 1], F32, kind="ExternalInput")
    b2 = nc.dram_tensor("b2", [KH, P, 1], F32, kind="ExternalInput")
    out = nc.dram_tensor("out", [O, C], F32, kind="ExternalOutput")

    # Balanced chunk widths (<= CHUNK each) so no chunk pays the small-N
    # per-matmul overhead premium.
    n_chunks = (C + CHUNK - 1) // CHUNK
    base, rem = divmod(C, n_chunks)
    widths = [base + (1 if i < rem else 0) for i in range(n_chunks)]
    starts = [sum(widths[:i]) for i in range(n_chunks)]
    relu = mybir.ActivationFunctionType.Relu

    with tile.TileContext(nc) as tc:
        with (
            tc.tile_pool(name="weights", bufs=1) as wp,
            tc.tile_pool(name="acts", bufs=1) as hp,
            tc.tile_pool(name="xin", bufs=2) as xp,
            tc.tile_pool(name="yout", bufs=3) as yp,
            tc.tile_pool(name="psum", bufs=2, space="PSUM") as pp,
        ):
            def dma_x(ci):
                t = xp.tile([P, KD, widths[ci]], BF16, tag="x")
                off = KD * starts[ci]
                nc.sync.dma_start(
                    t,
                    xF[:, off:off + KD * widths[ci]].rearrange(
                        "p (k c) -> p k c", k=KD
                    ),
                )
                return t

            # Short PE warmup on memset tiles: burns the cold HAM window
            # (half-rate PE clock) on dummy work before real data lands.
            wu_w = wp.tile([P, P], BF16, tag="wuw")
            wu_x = wp.tile([P, 256], BF16, tag="wux")
            nc.vector.memset(wu_w, 1.0)
            nc.vector.memset(wu_x, 1.0)
            wu_ps = pp.tile([P, 256], F32, tag="wups", bufs=1)
            for _ in range(44):
                nc.tensor.matmul(wu_ps, wu_w, wu_x, start=True, stop=True)

            # DMA issue order = consumption order: small biases and the
            # first chunks of x go first, then W1/W2/W3 k-tiles, so the
            # first matmuls start as soon as W1[k0] + x[0] land instead of
            # after the full 17 MB weight prologue.
            b1_dma = wp.tile([P, KH], F32, tag="b1d")
            nc.sync.dma_start(b1_dma, b1[:].rearrange("n p one -> p (n one)"))
            b2_dma = wp.tile([P, KH], F32, tag="b2d")
            nc.sync.dma_start(b2_dma, b2[:].rearrange("n p one -> p (n one)"))
            x_tiles = {0: dma_x(0)}
            # W1/W2 stream column-major (the order the ho/go loops consume
            # them) so chunk 0's compute never waits on a late k-tile.
            w1_k = [wp.tile([P, H], BF16, name=f"w1_{k}", tag=f"w1_{k}") for k in range(KD)]
            for k in range(KD):
                nc.sync.dma_start(w1_k[k], w1[k * P:(k + 1) * P, :])
            if n_chunks > 1:
                x_tiles[1] = dma_x(1)
            w2_k = [wp.tile([P, H], BF16, name=f"w2_{k}", tag=f"w2_{k}") for k in range(KH)]
            for k in range(KH):
                nc.sync.dma_start(w2_k[k], w2[k * P:(k + 1) * P, :])
            w3_k = []
            for k in range(KH):
                t = wp.tile([P, O], BF16, tag=f"w3_{k}")
                nc.sync.dma_start(t, w3[k * P:(k + 1) * P, :])
                w3_k.append(t)

            for ci in range(n_chunks):
                c0 = starts[ci]
                cw = widths[ci]
                x_sb = x_tiles.pop(ci)
                if ci + 2 < n_chunks:
                    x_tiles[ci + 2] = dma_x(ci + 2)
                h1 = hp.tile([P, KH, cw], BF16, tag="h1")
                for ho in range(KH):
                    ps = pp.tile([P, cw], F32, tag="ps1")
                    for k in range(KD):
                        nc.tensor.matmul(
                            ps,
                            w1_k[k][:, ho * P:(ho + 1) * P],
                            x_sb[:, k, :],
                            start=(k == 0),
                            stop=(k == KD - 1),
                        )
                    nc.scalar.activation(
                        h1[:, ho, :], ps, relu, bias=b1_dma[:, ho:ho + 1]
                    )
                h2 = hp.tile([P, KH, cw], BF16, tag="h2")
                for go in range(KH):
                    ps = pp.tile([P, cw], F32, tag="ps2")
                    for k in range(KH):
                        nc.tensor.matmul(
                            ps,
                            w2_k[k][:, go * P:(go + 1) * P],
                            h1[:, k, :],
                            start=(k == 0),
                            stop=(k == KH - 1),
                        )
                    nc.scalar.activation(
                        h2[:, go, :], ps, relu, bias=b2_dma[:, go:go + 1]
                    )
                for oo in range(KO):
                    ps = pp.tile([P, cw], F32, tag="ps3")
                    for k in range(KH):
                        nc.tensor.matmul(
                            ps,
                            w3_k[k][:, oo * P:(oo + 1) * P],
                            h2[:, k, :],
                            start=(k == 0),
                            stop=(k == KH - 1),
                        )
                    y = yp.tile([P, cw], F32, tag="y")
                    nc.vector.tensor_copy(y, ps)
                    nc.sync.dma_start(
                        out[oo * P:(oo + 1) * P, c0:c0 + cw], y
                    )
    nc.compile()
    return nc


def kernel(x, W1, b1, W2, b2, W3, b3, Wg, bg):
    global LAST_RESULT
    x = np.asarray(x, dtype=np.float32)
    xf = x.reshape(B * L, D)

    # Gate on host (routing): logits in f64 for a faithful top-2 selection.
    logits = xf.astype(np.float64) @ np.asarray(Wg, np.float64) + np.asarray(
        bg, np.float64
    )
    order = np.argsort(-logits, axis=1, kind="stable")[:, :K]
    topv = np.take_along_axis(logits, order, 1).astype(np.float32)
    mx = topv.max(1, keepdims=True)
    ex = np.exp(topv - mx)
    w = (ex / ex.sum(1, keepdims=True)).astype(np.float32)

    toks, wgts, counts = [], [], []
    for e in range(E):
        s0 = np.nonzero(order[:, 0] == e)[0]
        s1 = np.nonzero(order[:, 1] == e)[0]
        toks.append(np.concatenate([s0, s1]))
        wgts.append(np.concatenate([w[s0, 0], w[s1, 1]]).astype(np.float32))
        counts.append(len(toks[-1]))

    # Exact capacity: every core's kernel runs C token slots; shorter
    # experts are padded with token 0 and their extra rows dropped on the
    # host. Pad to a multiple of 4 to keep DMA rows 8-byte aligned.
    C = max(256, -(-max(counts) // 4) * 4)
    n_chunks = (C + CHUNK - 1) // CHUNK
    base, rem = divmod(C, n_chunks)
    widths = [base + (1 if i < rem else 0) for i in range(n_chunks)]
    starts = [sum(widths[:i]) for i in range(n_chunks)]
    nc = _nc_cache.get(C)
    if nc is None:
        nc = _build(C)
        _nc_cache[C] = nc

    bf = ml_dtypes.bfloat16
    in_maps = []
    for e in range(E):
        n = counts[e]
        pad_tok = np.zeros(C, np.int64)
        pad_tok[:n] = toks[e]
        gath = xf[pad_tok].astype(bf)  # [C, D]
        xFe = np.empty((P, KD * C), bf)
        for ci in range(n_chunks):
            c0, cw = starts[ci], widths[ci]
            blk = gath[c0:c0 + cw].T.reshape(KD, P, cw).transpose(1, 0, 2)
            xFe[:, KD * c0:KD * (c0 + cw)] = blk.reshape(P, KD * cw)
        in_maps.append({
            "xF": xFe,
            "w1": np.asarray(W1[e], np.float32).astype(bf),
            "w2": np.asarray(W2[e], np.float32).astype(bf),
            "w3": np.asarray(W3[e], np.float32).astype(bf),
            "b1": np.ascontiguousarray(
                np.asarray(b1[e], np.float32).reshape(KH, P, 1)
            ),
            "b2": np.ascontiguousarray(
                np.asarray(b2[e], np.float32).reshape(KH, P, 1)
            ),
        })

    res = run_bass_kernel_spmd(
        nc, in_maps, core_ids=list(range(N_CORES)), trace=TRACE
    )
    LAST_RESULT = res

    # Combine: weight each expert's [O, C] output by the token's gate weight
    # and scatter-add; every token has exactly K=2 entries.
    y_all = np.concatenate(
        [
            np.asarray(res.results[e]["out"])[:, :counts[e]].T * wgts[e][:, None]
            for e in range(E)
        ],
        axis=0,
    )
    idx_all = np.concatenate(toks)
    srt = np.argsort(idx_all, kind="stable")
    ys = y_all[srt]
    out_flat = ys[0::2] + ys[1::2]
    # b3 is applied here rather than on-device: out += sum_k w_k * b3[e_k]
    b3f = np.asarray(b3, np.float32)
    out_flat += w[:, 0, None] * b3f[order[:, 0]] + w[:, 1, None] * b3f[order[:, 1]]
    out = out_flat.reshape(B, L, O).astype(np.float32)

    usage = w.reshape(B, L, K).mean(axis=1)
    lbl = np.float32(usage.var(axis=-1, ddof=1).mean())
    return out, lbl
